# revision 1
# baseline (speedup 1.0000x reference)
"""Batched MoE (top-2, 8 experts) on 8 Trainium2 NeuronCores.

Strategy: expert-parallel — core e owns expert e's weights (w1/w2/w3) and
processes the tokens routed to it. Routing (sort by expert / capacity
padding) and the combine (weighting by gate prob + scatter-add over top-k)
are cheap O(tokens) index ops done on host; all matmul FLOPs run on device.

Device dataflow per core (capacity C columns, zero-padded):
    xt  = X_e^T               [1024, C]   (d on partitions)
    GT  = w1^T @ xt           [4096, C]   lhsT = w1 tiles (natural layout)
    VT  = w2^T @ xt           [4096, C]
    HT  = silu(GT) * VT       [4096, C]
    OT  = w3^T @ HT           [1024, C]   lhsT = w3 tiles (natural layout)
All operands enter the PE in their natural DRAM layout — no transposes.
Matmuls run in float32r (TF32-class, 1 cycle/row at free-dim >= 256, 4x
faster than plain fp32). The f dimension is processed in chunks of 512 so
weights stream through SBUF; OT accumulates across chunks in SBUF via DVE.
Phase B (OT accumulation) of chunk ch is issued after phase A of chunk
ch+1 so the PE never waits on the ACT/DVE epilogue that produces HT.
"""

import numpy as np

N_EXPERTS = 8
D_MODEL = 1024
D_FF = 4096
# d_ff chunk sizes streamed through SBUF
CHUNKS = [512] * 8
KT = D_MODEL // 128     # 8 k-tiles (contraction d)
MT = D_MODEL // 128     # 8 output d-tiles

_program_cache = {}


def _col_chunks(C):
    """Split C columns into <=512 pieces, all >=256 when possible (float32r
    runs at 1 cycle/row only for free dim >= 256)."""
    if C <= 512:
        return [(0, C)]
    n = (C + 511) // 512
    base = C // n
    rem = C - base * n
    out = []
    off = 0
    for i in range(n):
        sz = base + (1 if i < rem else 0)
        out.append((off, sz))
        off += sz
    return out


def _build_program(C):
    import concourse.bacc as bacc
    import concourse.mybir as mybir
    from concourse.tile import TileContext

    F32R = mybir.dt.float32r
    F32 = mybir.dt.float32
    SILU = mybir.ActivationFunctionType.Silu
    ccs = _col_chunks(C)

    nc = bacc.Bacc()
    xt_d = nc.declare_dram_parameter("xt", [D_MODEL, C], F32R, isOutput=False)
    w1_d = nc.declare_dram_parameter("w1", [D_MODEL, D_FF], F32R, isOutput=False)
    w2_d = nc.declare_dram_parameter("w2", [D_MODEL, D_FF], F32R, isOutput=False)
    w3_d = nc.declare_dram_parameter("w3", [D_FF, D_MODEL], F32R, isOutput=False)
    ot_d = nc.declare_dram_parameter("ot", [D_MODEL, C], F32, isOutput=True)

    xt_r = xt_d.rearrange("(k p) c -> k p c", p=128)
    w1_r = w1_d.rearrange("(k p) f -> k p f", p=128)
    w2_r = w2_d.rearrange("(k p) f -> k p f", p=128)
    w3_r = w3_d.rearrange("(j p) d -> j p d", p=128)
    ot_r = ot_d.rearrange("(m p) c -> m p c", p=128)

    NCH = len(CHUNKS)
    f_offs = [sum(CHUNKS[:i]) for i in range(NCH)]

    with TileContext(nc) as tc:
        with (
            tc.tile_pool(name="xtp", bufs=1) as xt_pool,
            tc.tile_pool(name="w12", bufs=2) as w12_pool,
            tc.tile_pool(name="w3p", bufs=2) as w3_pool,
            tc.tile_pool(name="htp", bufs=2) as ht_pool,
            tc.tile_pool(name="otp", bufs=1) as ot_pool,
            tc.tile_pool(name="tmp", bufs=4) as tmp_pool,
            tc.tile_pool(name="pg", bufs=2, space="PSUM") as pg_pool,
            tc.tile_pool(name="pv", bufs=2, space="PSUM") as pv_pool,
            tc.tile_pool(name="po", bufs=3, space="PSUM") as po_pool,
        ):
            xt_sb = [None] * KT
            ot_sb = [
                ot_pool.tile([128, C], F32, tag=f"ot{m}", name=f"ot{m}")
                for m in range(MT)
            ]

            def load_w1(ch):
                f0, fc = f_offs[ch], CHUNKS[ch]
                w1c = []
                for k in range(KT):
                    t1 = w12_pool.tile(
                        [128, fc], F32R, tag=f"w1k{k}", name=f"w1c{k}"
                    )
                    if ch == 0:
                        # xt slab k lands immediately before the w1 slab the
                        # same-k matmul needs; one queue in consumption order
                        # beats two contending rings during the ramp
                        xt_t = xt_pool.tile(
                            [128, C], F32R, tag=f"xt{k}", name=f"xt{k}"
                        )
                        nc.sync.dma_start(out=xt_t[:], in_=xt_r[k])
                        xt_sb[k] = xt_t
                    nc.sync.dma_start(out=t1[:], in_=w1_r[k][:, f0 : f0 + fc])
                    w1c.append(t1)
                return w1c

            def load_w2(ch):
                f0, fc = f_offs[ch], CHUNKS[ch]
                w2c = []
                for k in range(KT):
                    t2 = w12_pool.tile(
                        [128, fc], F32R, tag=f"w2k{k}", name=f"w2c{k}"
                    )
                    nc.sync.dma_start(out=t2[:], in_=w2_r[k][:, f0 : f0 + fc])
                    w2c.append(t2)
                return w2c

            def load_w3(ch):
                f0, fc = f_offs[ch], CHUNKS[ch]
                w3c = []
                for jj in range(fc // 128):
                    t3 = w3_pool.tile(
                        [128, D_MODEL], F32R, tag=f"w3j{jj % 4}", name=f"w3c{jj}"
                    )
                    nc.sync.dma_start(out=t3[:], in_=w3_r[f0 // 128 + jj])
                    w3c.append(t3)
                return w3c

            def load_chunk(ch):
                w1c = load_w1(ch)
                w2c = load_w2(ch)
                w3c = load_w3(ch)
                return w1c, w2c, w3c

            def phase_a(ch, w1c, w2c):
                """GT/VT matmuls + silu*mul epilogue -> HT tiles for a chunk."""
                jt = CHUNKS[ch] // 128
                hts = []
                for jj in range(jt):
                    ht_t = ht_pool.tile(
                        [128, C], F32R, tag=f"ht{jj % 4}", name=f"ht{jj}"
                    )
                    js = slice(jj * 128, (jj + 1) * 128)
                    for c0, cl in ccs:
                        cs = slice(c0, c0 + cl)
                        pg = pg_pool.tile([128, cl], F32, tag="pg", name="pg")
                        pv = pv_pool.tile([128, cl], F32, tag="pv", name="pv")
                        for k in range(KT):
                            nc.tensor.matmul(
                                out=pg[:],
                                lhsT=w1c[k][:, js],
                                rhs=xt_sb[k][:, cs],
                                start=(k == 0),
                                stop=(k == KT - 1),
                            )
                        for k in range(KT):
                            nc.tensor.matmul(
                                out=pv[:],
                                lhsT=w2c[k][:, js],
                                rhs=xt_sb[k][:, cs],
                                start=(k == 0),
                                stop=(k == KT - 1),
                            )
                        st = tmp_pool.tile([128, cl], F32, tag="silu", name="st")
                        nc.scalar.activation(st[:], pg[:], SILU)
                        nc.vector.tensor_mul(out=ht_t[:, cs], in0=st[:], in1=pv[:])
                    hts.append(ht_t)
                return hts

            def phase_b_m(ch, w3c, hts, m):
                """OT partial accumulation for one output d-tile of a chunk."""
                jt = len(hts)
                ms = slice(m * 128, (m + 1) * 128)
                for c0, cl in ccs:
                    cs = slice(c0, c0 + cl)
                    po = po_pool.tile([128, cl], F32, tag="po", name="po")
                    for jj in range(jt):
                        nc.tensor.matmul(
                            out=po[:],
                            lhsT=w3c[jj][:, ms],
                            rhs=hts[jj][:, cs],
                            start=(jj == 0),
                            stop=(jj == jt - 1),
                        )
                    if ch == 0:
                        nc.vector.tensor_copy(out=ot_sb[m][:, cs], in_=po[:])
                    else:
                        nc.vector.tensor_add(
                            out=ot_sb[m][:, cs], in0=ot_sb[m][:, cs], in1=po[:]
                        )

            def phase_b(ch, w3c, hts):
                for m in range(MT):
                    phase_b_m(ch, w3c, hts, m)

            # software pipeline: B(ch) issues after A(ch+1) so phase B never
            # stalls the PE on the ACT/DVE epilogue producing its HT input.
            # DMAs are emitted in exact PE consumption order — w3(ch-1) goes
            # out after w1/w2(ch), matching the A(ch) -> B(ch-1) issue order,
            # so each chunk's w3 never delays the w1/w2 the PE needs first.
            # The last two B passes interleave m-wise so each OT slab's store
            # DMA overlaps the remaining matmuls instead of draining at the
            # very end.
            w1c, w2c = load_w1(0), load_w2(0)
            hts_prev = phase_a(0, w1c, w2c)
            w3_prev = None
            for ch in range(1, NCH):
                w1c, w2c = load_w1(ch), load_w2(ch)
                w3_prev = load_w3(ch - 1)
                hts = phase_a(ch, w1c, w2c)
                if ch < NCH - 1:
                    phase_b(ch - 1, w3_prev, hts_prev)
                    hts_prev = hts
            w3_last = load_w3(NCH - 1)
            for m in range(MT):
                phase_b_m(NCH - 2, w3_prev, hts_prev, m)
                phase_b_m(NCH - 1, w3_last, hts, m)
                nc.sync.dma_start(out=ot_r[m], in_=ot_sb[m][:])

    nc.compile()
    return nc


def _get_program(C):
    if C not in _program_cache:
        _program_cache[C] = _build_program(C)
    return _program_cache[C]


def _run(nc, in_maps, trace=False):
    import time

    from concourse.bass_utils import run_bass_kernel_spmd

    last = None
    for attempt in range(4):
        try:
            return run_bass_kernel_spmd(
                nc, in_maps, list(range(N_EXPERTS)), trace=trace
            )
        except Exception as e:  # stale device state from a prior crashed run
            last = e
            time.sleep(10 * (attempt + 1))
            try:  # poke the runtime with a trivial op to clear/verify state
                import jax
                import jax.numpy as jnp

                jnp.add(jnp.ones((8, 8)), 1.0).block_until_ready()
            except Exception:
                pass
    raise last


def kernel(x, expert_indices, expert_weights, w1, w2, w3, _trace=False):
    x = np.ascontiguousarray(np.asarray(x, dtype=np.float32))
    expert_indices = np.asarray(expert_indices)
    expert_weights = np.asarray(expert_weights, dtype=np.float32)
    w1 = np.asarray(w1, dtype=np.float32)
    w2 = np.asarray(w2, dtype=np.float32)
    w3 = np.asarray(w3, dtype=np.float32)

    n_tokens, d_model = x.shape
    top_k = expert_indices.shape[1]
    n_experts = w1.shape[0]
    A = n_tokens * top_k

    flat_e = expert_indices.reshape(-1).astype(np.int64)
    flat_w = expert_weights.reshape(-1)
    tok_idx = np.repeat(np.arange(n_tokens), top_k)
    order = np.argsort(flat_e, kind="stable")
    s_tok = tok_idx[order]
    s_w = flat_w[order]
    counts = np.bincount(flat_e, minlength=n_experts)
    starts = np.concatenate([[0], np.cumsum(counts)[:-1]])

    C = int(counts.max())
    C = max(256, -(-C // 4) * 4)  # round up to multiple of 4 (16B rows)

    xt = np.zeros((n_experts, d_model, C), np.float32)
    for e in range(n_experts):
        seg = s_tok[starts[e] : starts[e] + counts[e]]
        xt[e, :, : counts[e]] = x[seg].T

    nc = _get_program(C)
    in_maps = [
        {"xt": xt[e], "w1": w1[e], "w2": w2[e], "w3": w3[e]}
        for e in range(n_experts)
    ]
    res = _run(nc, in_maps, trace=_trace)

    y = np.empty((A, d_model), np.float32)
    for e in range(n_experts):
        ot = res.results[e]["ot"]
        y[starts[e] : starts[e] + counts[e]] = ot[:, : counts[e]].T
    y *= s_w[:, None]
    y_orig = np.empty_like(y)
    y_orig[order] = y
    out = y_orig.reshape(n_tokens, top_k, d_model).sum(axis=1, dtype=np.float32)
    if _trace:
        return out.astype(np.float32, copy=False), res
    return out.astype(np.float32, copy=False)



# revision 2
# speedup vs baseline: 1.0613x; 1.0613x over previous
"""Batched MoE (top-2, 8 experts) on 8 Trainium2 NeuronCores.

Strategy: expert-parallel — core e owns expert e's weights (w1/w2/w3) and
processes the tokens routed to it. Routing (sort by expert / capacity
padding) and the combine (weighting by gate prob + scatter-add over top-k)
are cheap O(tokens) index ops done on host; all matmul FLOPs run on device.

Device dataflow per core (capacity C columns, zero-padded):
    xt  = X_e^T               [1024, C]   (d on partitions)
    GT  = w1^T @ xt           [4096, C]   lhsT = w1 tiles (natural layout)
    VT  = w2^T @ xt           [4096, C]
    HT  = silu(GT) * VT       [4096, C]
    OT  = w3^T @ HT           [1024, C]   lhsT = w3 tiles (natural layout)
All operands enter the PE in their natural DRAM layout — no transposes.
All matmul operands are bf16 (1 cycle/row on the PE, same as fp32r, but
half the HBM traffic — the fp32 version was DMA-limited at ~90% of the
per-core HBM bandwidth and stalled the PE). PSUM accumulation is fp32;
the OT accumulator in SBUF is fp32; only the final chunk's add converts
to bf16 for the store. The f dimension is processed in chunks of 512 so
weights stream through SBUF. Phase B (OT accumulation) of chunk ch is
issued after phase A of chunk ch+1 so the PE never waits on the ACT/DVE
epilogue that produces HT.
"""

import numpy as np

N_EXPERTS = 8
D_MODEL = 1024
D_FF = 4096
# d_ff chunk sizes streamed through SBUF
CHUNKS = [512] * 8
KT = D_MODEL // 128     # 8 k-tiles (contraction d)
MT = D_MODEL // 128     # 8 output d-tiles

_program_cache = {}


def _col_chunks(C):
    """Split C columns into <=512 pieces (PSUM bank limit for fp32 out)."""
    if C <= 512:
        return [(0, C)]
    n = (C + 511) // 512
    base = C // n
    rem = C - base * n
    out = []
    off = 0
    for i in range(n):
        sz = base + (1 if i < rem else 0)
        out.append((off, sz))
        off += sz
    return out


def _build_program(C):
    import concourse.bacc as bacc
    import concourse.mybir as mybir
    from concourse.tile import TileContext

    BF16 = mybir.dt.bfloat16
    F32 = mybir.dt.float32
    SILU = mybir.ActivationFunctionType.Silu
    ccs = _col_chunks(C)

    nc = bacc.Bacc()
    xt_d = nc.declare_dram_parameter("xt", [D_MODEL, C], BF16, isOutput=False)
    w1_d = nc.declare_dram_parameter("w1", [D_MODEL, D_FF], BF16, isOutput=False)
    w2_d = nc.declare_dram_parameter("w2", [D_MODEL, D_FF], BF16, isOutput=False)
    w3_d = nc.declare_dram_parameter("w3", [D_FF, D_MODEL], BF16, isOutput=False)
    ot_d = nc.declare_dram_parameter("ot", [D_MODEL, C], BF16, isOutput=True)

    xt_r = xt_d.rearrange("(k p) c -> k p c", p=128)
    w1_r = w1_d.rearrange("(k p) f -> k p f", p=128)
    w2_r = w2_d.rearrange("(k p) f -> k p f", p=128)
    w3_r = w3_d.rearrange("(j p) d -> j p d", p=128)
    ot_r = ot_d.rearrange("(m p) c -> m p c", p=128)

    NCH = len(CHUNKS)
    f_offs = [sum(CHUNKS[:i]) for i in range(NCH)]

    with TileContext(nc) as tc:
        with (
            tc.tile_pool(name="xtp", bufs=1) as xt_pool,
            tc.tile_pool(name="w12", bufs=2) as w12_pool,
            tc.tile_pool(name="w3p", bufs=2) as w3_pool,
            tc.tile_pool(name="htp", bufs=2) as ht_pool,
            tc.tile_pool(name="otp", bufs=1) as ot_pool,
            tc.tile_pool(name="ot16", bufs=1) as ot16_pool,
            tc.tile_pool(name="tmp", bufs=4) as tmp_pool,
            tc.tile_pool(name="pg", bufs=2, space="PSUM") as pg_pool,
            tc.tile_pool(name="pv", bufs=2, space="PSUM") as pv_pool,
            tc.tile_pool(name="po", bufs=3, space="PSUM") as po_pool,
        ):
            xt_sb = [None] * KT
            ot_sb = [
                ot_pool.tile([128, C], F32, tag=f"ot{m}", name=f"ot{m}")
                for m in range(MT)
            ]
            ot16_sb = [
                ot16_pool.tile([128, C], BF16, tag=f"o16{m}", name=f"o16{m}")
                for m in range(MT)
            ]

            def load_w1(ch):
                f0, fc = f_offs[ch], CHUNKS[ch]
                w1c = []
                for k in range(KT):
                    t1 = w12_pool.tile(
                        [128, fc], BF16, tag=f"w1k{k}", name=f"w1c{k}"
                    )
                    if ch == 0:
                        # xt slab k lands immediately before the w1 slab the
                        # same-k matmul needs; split into col halves so the
                        # slab spreads over two DMA queues (halves the time
                        # until the first matmul group is runnable)
                        xt_t = xt_pool.tile(
                            [128, C], BF16, tag=f"xt{k}", name=f"xt{k}"
                        )
                        for c0, cl in ccs:
                            nc.sync.dma_start(
                                out=xt_t[:, c0 : c0 + cl],
                                in_=xt_r[k][:, c0 : c0 + cl],
                            )
                        xt_sb[k] = xt_t
                    nc.sync.dma_start(out=t1[:], in_=w1_r[k][:, f0 : f0 + fc])
                    w1c.append(t1)
                return w1c

            def load_w2(ch):
                f0, fc = f_offs[ch], CHUNKS[ch]
                w2c = []
                for k in range(KT):
                    t2 = w12_pool.tile(
                        [128, fc], BF16, tag=f"w2k{k}", name=f"w2c{k}"
                    )
                    nc.sync.dma_start(out=t2[:], in_=w2_r[k][:, f0 : f0 + fc])
                    w2c.append(t2)
                return w2c

            def load_w3(ch):
                f0, fc = f_offs[ch], CHUNKS[ch]
                w3c = []
                for jj in range(fc // 128):
                    t3 = w3_pool.tile(
                        [128, D_MODEL], BF16, tag=f"w3j{jj % 4}", name=f"w3c{jj}"
                    )
                    nc.sync.dma_start(out=t3[:], in_=w3_r[f0 // 128 + jj])
                    w3c.append(t3)
                return w3c

            def phase_a(ch, w1c, w2c):
                """GT/VT matmuls + silu*mul epilogue -> HT tiles for a chunk."""
                jt = CHUNKS[ch] // 128
                hts = []
                for jj in range(jt):
                    ht_t = ht_pool.tile(
                        [128, C], BF16, tag=f"ht{jj % 4}", name=f"ht{jj}"
                    )
                    js = slice(jj * 128, (jj + 1) * 128)
                    for c0, cl in ccs:
                        cs = slice(c0, c0 + cl)
                        pg = pg_pool.tile([128, cl], F32, tag="pg", name="pg")
                        pv = pv_pool.tile([128, cl], F32, tag="pv", name="pv")
                        for k in range(KT):
                            nc.tensor.matmul(
                                out=pg[:],
                                lhsT=w1c[k][:, js],
                                rhs=xt_sb[k][:, cs],
                                start=(k == 0),
                                stop=(k == KT - 1),
                            )
                        for k in range(KT):
                            nc.tensor.matmul(
                                out=pv[:],
                                lhsT=w2c[k][:, js],
                                rhs=xt_sb[k][:, cs],
                                start=(k == 0),
                                stop=(k == KT - 1),
                            )
                        st = tmp_pool.tile([128, cl], F32, tag="silu", name="st")
                        nc.scalar.activation(st[:], pg[:], SILU)
                        nc.vector.tensor_mul(out=ht_t[:, cs], in0=st[:], in1=pv[:])
                    hts.append(ht_t)
                return hts

            def phase_b_m(ch, w3c, hts, m):
                """OT partial accumulation for one output d-tile of a chunk."""
                jt = len(hts)
                ms = slice(m * 128, (m + 1) * 128)
                for c0, cl in ccs:
                    cs = slice(c0, c0 + cl)
                    po = po_pool.tile([128, cl], F32, tag="po", name="po")
                    for jj in range(jt):
                        nc.tensor.matmul(
                            out=po[:],
                            lhsT=w3c[jj][:, ms],
                            rhs=hts[jj][:, cs],
                            start=(jj == 0),
                            stop=(jj == jt - 1),
                        )
                    if ch == 0:
                        nc.vector.tensor_copy(out=ot_sb[m][:, cs], in_=po[:])
                    elif ch == NCH - 1:
                        # final accumulation converts to bf16 for the store
                        nc.vector.tensor_add(
                            out=ot16_sb[m][:, cs], in0=ot_sb[m][:, cs], in1=po[:]
                        )
                    else:
                        nc.vector.tensor_add(
                            out=ot_sb[m][:, cs], in0=ot_sb[m][:, cs], in1=po[:]
                        )

            def phase_b(ch, w3c, hts):
                for m in range(MT):
                    phase_b_m(ch, w3c, hts, m)

            # software pipeline: B(ch) issues after A(ch+1) so phase B never
            # stalls the PE on the ACT/DVE epilogue producing its HT input.
            # DMAs are emitted in exact PE consumption order — w3(ch-1) goes
            # out after w1/w2(ch), matching the A(ch) -> B(ch-1) issue order,
            # so each chunk's w3 never delays the w1/w2 the PE needs first.
            # The last two B passes interleave m-wise so each OT slab's store
            # DMA overlaps the remaining matmuls instead of draining at the
            # very end; stores go out in col-chunk pieces to spread queues.
            w1c, w2c = load_w1(0), load_w2(0)
            hts_prev = phase_a(0, w1c, w2c)
            w3_prev = None
            for ch in range(1, NCH):
                w1c, w2c = load_w1(ch), load_w2(ch)
                w3_prev = load_w3(ch - 1)
                hts = phase_a(ch, w1c, w2c)
                if ch < NCH - 1:
                    phase_b(ch - 1, w3_prev, hts_prev)
                    hts_prev = hts
            w3_last = load_w3(NCH - 1)
            for m in range(MT):
                phase_b_m(NCH - 2, w3_prev, hts_prev, m)
                phase_b_m(NCH - 1, w3_last, hts, m)
                for c0, cl in ccs:
                    nc.sync.dma_start(
                        out=ot_r[m][:, c0 : c0 + cl],
                        in_=ot16_sb[m][:, c0 : c0 + cl],
                    )

    nc.compile()
    return nc


def _get_program(C):
    if C not in _program_cache:
        _program_cache[C] = _build_program(C)
    return _program_cache[C]


def _run(nc, in_maps, trace=False):
    import time

    from concourse.bass_utils import run_bass_kernel_spmd

    last = None
    for attempt in range(4):
        try:
            return run_bass_kernel_spmd(
                nc, in_maps, list(range(N_EXPERTS)), trace=trace
            )
        except Exception as e:  # stale device state from a prior crashed run
            last = e
            time.sleep(10 * (attempt + 1))
            try:  # poke the runtime with a trivial op to clear/verify state
                import jax
                import jax.numpy as jnp

                jnp.add(jnp.ones((8, 8)), 1.0).block_until_ready()
            except Exception:
                pass
    raise last


def kernel(x, expert_indices, expert_weights, w1, w2, w3, _trace=False):
    import ml_dtypes

    BF16 = ml_dtypes.bfloat16

    x = np.ascontiguousarray(np.asarray(x, dtype=np.float32))
    expert_indices = np.asarray(expert_indices)
    expert_weights = np.asarray(expert_weights, dtype=np.float32)
    w1 = np.asarray(w1, dtype=np.float32)
    w2 = np.asarray(w2, dtype=np.float32)
    w3 = np.asarray(w3, dtype=np.float32)

    n_tokens, d_model = x.shape
    top_k = expert_indices.shape[1]
    n_experts = w1.shape[0]
    A = n_tokens * top_k

    flat_e = expert_indices.reshape(-1).astype(np.int64)
    flat_w = expert_weights.reshape(-1)
    tok_idx = np.repeat(np.arange(n_tokens), top_k)
    order = np.argsort(flat_e, kind="stable")
    s_tok = tok_idx[order]
    s_w = flat_w[order]
    counts = np.bincount(flat_e, minlength=n_experts)
    starts = np.concatenate([[0], np.cumsum(counts)[:-1]])

    C = int(counts.max())
    C = max(256, -(-C // 4) * 4)  # round up to multiple of 4 (8B bf16 rows)

    w1_16 = w1.astype(BF16)
    w2_16 = w2.astype(BF16)
    w3_16 = w3.astype(BF16)
    xt = np.zeros((n_experts, d_model, C), BF16)
    for e in range(n_experts):
        seg = s_tok[starts[e] : starts[e] + counts[e]]
        xt[e, :, : counts[e]] = x[seg].astype(BF16).T

    nc = _get_program(C)
    in_maps = [
        {"xt": xt[e], "w1": w1_16[e], "w2": w2_16[e], "w3": w3_16[e]}
        for e in range(n_experts)
    ]
    res = _run(nc, in_maps, trace=_trace)

    y = np.empty((A, d_model), np.float32)
    for e in range(n_experts):
        ot = np.asarray(res.results[e]["ot"]).astype(np.float32)
        y[starts[e] : starts[e] + counts[e]] = ot[:, : counts[e]].T
    y *= s_w[:, None]
    y_orig = np.empty_like(y)
    y_orig[order] = y
    out = y_orig.reshape(n_tokens, top_k, d_model).sum(axis=1, dtype=np.float32)
    if _trace:
        return out.astype(np.float32, copy=False), res
    return out.astype(np.float32, copy=False)
